# revision 16
# baseline (speedup 1.0000x reference)
"""Binary conv2d (XNOR-style) + per-channel scale for Trainium2 — v3.

y = conv2d(sign(x), sign(w), stride=1, pad=1) * scale[oc]

Data-parallel over batch across 8 NeuronCores (4 images each).  The 3x3
conv over 256 in-channels is accumulating fp8 DoubleRow matmuls (K=256)
into a PSUM tile per 8-output-row chunk, using shifted windows of a
zero-padded 57-column-stride image.  PSUM accumulates in fp32 and all
matmul inputs are exactly representable, so the result is bit-identical
to the fp32 reference.

The matmul stream is at ~97% of the fp8-DR peak (195 ns issue-to-issue
vs the 190 ns floor of 456 cycles @2.4 GHz), so everything here is about
starting that stream as early as possible and never letting it gap
(a PE idle gap also resets the 2.4 GHz clock to 1.2 GHz for ~3 us).

Hardware facts this layout is built around (measured via NTFF traces):
  - DMA transfers progress CONCURRENTLY and fairly: N in-flight
    transfers each get ~1/N of the ~390 GB/s, so issuing everything up
    front makes the critical first piece land late.  Phase A therefore
    carries ONLY {scale, 9 input rows (both ic slots), all weights}
    (~1.1 MB, all landed by ~10.5 us); every later piece's dma_start is
    held back on the DVE queue behind a tiny tensor_copy "dummy read"
    of the previous piece, which completes exactly when that piece's
    data lands (pool-reader or engine-order pacing would wait for the
    SIGN instead, serializing transfer behind compute).
  - The PE clock ramps 1.2 -> 2.4 GHz only after ~3 us of continuous
    HIGH-OCCUPANCY execution: K=2 warmup matmuls never ramp it (the
    governor watches array activity), so the warmup runs K=256
    DoubleRow matmuls with a 256-wide free dim (107 ns each when
    ramped) off a [128,2,592] zero scratch whose memset is split
    across gpsimd+vector to finish by ~8.5 us.  The warmup count is
    sized so the last warmup retires right as piece-0's sign completes.
  - Image 0 is signed in four pieces (9/16/16/15 rows, both ic slots in
    one ACT sign) and each fine-grained compute group is emitted right
    after the one sign it needs: (0,1)<-P0, (1,3)<-P1, (3,5)<-P2,
    (5,7)<-P3, so dependency tracking cannot over-wait on later signs.
  - Images 1-3 load through the scalar ring; ACT engine order (their
    issues are emitted after image 0's last sign) plus the xin pool
    rotation paces them off the startup-critical window.
"""

import numpy as np
import ml_dtypes

N_CORES = 8
IMGS = 4  # images per core
IC = 256
OC = 256
H = W = 56
# Padded row stride is 57, not 58: for a 3-wide kernel the left pad of
# row r+1 doubles as the right pad of row r, halving the dead columns.
WPAD = 57
XPAD_F = 3312  # 58 padded rows * 57 = 3306 -> pad to mult of 16
ROWS = 8  # output rows per PSUM tile
NFREE = ROWS * WPAD  # 456 <= 512 (PSUM bank limit)
NCHUNK = H // ROWS  # 7
# image-0 piece row ranges: piece k covers input rows through 8k+8, which
# is exactly what chunk k (output rows 8k..8k+7) needs
PR0 = tuple((max(0, 8 * k + 1) if k else 0, 8 * k + 9) for k in range(NCHUNK))
PR0 = tuple((a, min(b, H)) for a, b in PR0)  # last piece ends at row 56
# images 1-3 load in coarser pieces (ACT-queue budget)
PR = ((0, 9), (9, 25), (25, 41), (41, 56))
PMAX = 16  # max piece rows (pool tiles are uniform)
N_WARM = 24  # PE-clock warmup matmuls (256-wide): ~3us ramp + land at data-ready
WFREE = 256  # warmup matmul free dim

_cache = {}


def _install_drain_patch():
    """This walrus build rejects >1 sync-wait on ctrl-type instructions;
    Tile's kernel-tail drain carries one wait per pending proc.  Split it
    into one drain per proc (each with <=1 wait)."""
    import concourse.tile as _tile
    from concourse.vector_clock import ScopedClock, VectorClock

    if getattr(_tile.TileContext, "_drain_split_patch", False):
        return

    def _drain_and_barrier(self, tick_clock, wait_clock):
        nc = self.nc
        gclock = tick_clock.global_clock
        n = len(gclock)
        for p in range(n):
            t = gclock[p]
            if t <= 0:
                continue
            vec = [0] * n
            vec[p] = t
            d = nc.gpsimd.drain()
            wait_clock.add_sem_waits(d.ins, ScopedClock({None: VectorClock(vec)}))
        assert self.sems is not None
        popped = nc._tile_sem_poison_stack.pop()
        assert popped is self._sem_poison
        nc.clear_and_free_semaphores(list(self.sems.allocated().values()))

    _tile.TileContext._drain_and_barrier = _drain_and_barrier
    _tile.TileContext._drain_split_patch = True


def _split_excess_waits(nc, maxw=1):
    """Same walrus limitation: hoist excess sync-waits onto same-engine
    NoOps inserted just before the instruction (engine streams are
    in-order, so a preceding NoOp carrying the waits is equivalent)."""
    import concourse.mybir as mybir

    n_split = 0
    for f in nc.m.functions:
        for bb in f.blocks:
            out = []
            for ins in bb.instructions:
                si = ins.sync_info
                if si and si.on_wait and len(si.on_wait) > maxw:
                    waits = list(si.on_wait)
                    excess, keep = waits[:-maxw], waits[-maxw:]
                    for i in range(0, len(excess), maxw):
                        nop = mybir.InstNoOp(
                            name=f"{ins.name}_waitsplit{i}",
                            engine=ins.engine,
                            ins=[],
                            outs=[],
                            sync_info=mybir.SyncInfo(
                                on_wait=excess[i : i + maxw], on_update=[]
                            ),
                        )
                        out.append(nop)
                    si.on_wait = keep
                    n_split += 1
                out.append(ins)
            bb.instructions = out
    return n_split


def build_nc():
    import concourse.bass as bass
    import concourse.mybir as mybir
    from concourse.tile import TileContext

    _install_drain_patch()

    f32 = mybir.dt.float32
    fp8 = mybir.dt.float8e4
    DR = mybir.MatmulPerfMode.DoubleRow

    nc = bass.Bass()
    x = nc.declare_dram_parameter("x", [IMGS, IC, H, W], f32, isOutput=False)
    wb8 = nc.declare_dram_parameter("wb8", [128, 18, OC], fp8, isOutput=False)
    sc2 = nc.declare_dram_parameter("sc2", [128, 2], f32, isOutput=False)
    y = nc.declare_dram_parameter("y", [IMGS, OC, H, W], f32, isOutput=True)

    with TileContext(nc) as tc:
        with (
            tc.tile_pool(name="const", bufs=1) as cpool,
            tc.tile_pool(name="xin0", bufs=len(PR0)) as p0_pool,
            tc.tile_pool(name="xin", bufs=4) as xin_pool,
            tc.tile_pool(name="outp", bufs=6) as out_pool,
            tc.tile_pool(name="psum", bufs=8, space="PSUM") as psum_pool,
        ):
            wb = cpool.tile([128, 18, OC], fp8)
            sc = cpool.tile([128, 2], f32)
            xp = cpool.tile([128, IMGS * 2, XPAD_F], fp8)
            wsc = cpool.tile([128, 2, 592], fp8)  # warmup scratch

            # warmup scratch memset split across the two idle engines so
            # the first warmup matmul can issue by ~8.5us
            nc.gpsimd.memset(wsc[:, :, 0:296], 0.0)
            nc.vector.memset(wsc[:, :, 296:592], 0.0)

            def piece_tile(n, p):
                return xin_pool.tile(
                    [128, 2, PMAX, W], f32, name=f"xin{n}_{p}", tag="xin"
                )

            # image 0's 7 pieces are all live until signed — own pool
            p_tiles = [
                p0_pool.tile([128, 2, 9, W], f32, name=f"xin0_{p}", tag="xin0")
                for p in range(len(PR0))
            ]

            # --- phase A: ONLY the startup-critical bytes (sc + piece 0
            # + the weights in two chunks, ~1.1MB): in-flight DMAs share
            # bandwidth fairly, so every extra concurrent MB delays the
            # critical piece-0 landing by ~2.6us.
            (r0, r1) = PR0[0]
            nc.sync.dma_start(
                out=p_tiles[0][:, 0, 0 : r1 - r0, :], in_=x[0, 0:128, r0:r1, :]
            )
            nc.scalar.dma_start(out=sc[:], in_=sc2[:, :])
            nc.scalar.dma_start(
                out=p_tiles[0][:, 1, 0 : r1 - r0, :], in_=x[0, 128:256, r0:r1, :]
            )
            nc.sync.dma_start(out=wb[:, 0:8, :], in_=wb8[:, 0:8, :])
            nc.sync.dma_start(out=wb[:, 8:18, :], in_=wb8[:, 8:18, :])

            # --- PE clock warmup: K=256 DoubleRow matmuls (low-K matmuls
            # do NOT ramp the clock) with a 256-wide free dim so the
            # warmup tail quantizes finely (107ns each once ramped).
            for k in range(N_WARM):
                ps = psum_pool.tile([128, NFREE], f32, name=f"warm{k}", tag="ps")
                nc.tensor.matmul(
                    ps[:, 0:WFREE], wsc[:, :, 456:584], wsc[:, :, 0:WFREE],
                    start=True, stop=True, perf_mode=DR,
                )

            def pad_ring(j):
                # zero only the padding ring (interior is overwritten by
                # the sign): top pad row; each data row's col 0 (also the
                # previous row's right pad); bottom pad row + tail.
                eng = nc.vector if j % 2 == 0 else nc.gpsimd
                xpj = xp[:, j, :]
                eng.memset(xpj[:, 0:WPAD], 0.0)
                lefts = xpj[:, WPAD : WPAD + H * WPAD].rearrange(
                    "p (r c) -> p r c", c=WPAD
                )[:, :, 0:1]
                eng.memset(lefts, 0.0)
                eng.memset(xpj[:, (H + 1) * WPAD : XPAD_F], 0.0)

            pad_ring(0)
            pad_ring(1)

            def sign_piece(n, p, tile, ranges):
                # binarize both ic-slots of one piece to +-1 via the ACT
                # sign activation (signs own ACT; drains own DVE).
                r0, r1 = ranges[p]
                base = (r0 + 1) * WPAD + 1
                dst = (
                    xp[:, 2 * n : 2 * n + 2, base : base + (r1 - r0) * WPAD]
                    .rearrange("p j (h w) -> p j h w", w=WPAD)[:, :, :, 0:W]
                )
                nc.scalar.sign(dst, tile[:, :, 0 : r1 - r0, :])

            def compute_image(n, subs):
                # tap-outer (weight-stationary) so consecutive matmuls hit
                # different PSUM banks.  LDWEIGHTS overlaps MATMUL via the
                # PE dual weight buffer.
                for c0, c1 in subs:
                    for ocb in range(2):
                        psums = [
                            psum_pool.tile(
                                [128, NFREE], f32, name=f"ps{n}{ocb}{c}", tag="ps"
                            )
                            for c in range(c0, c1)
                        ]
                        for t in range(9):
                            kh, kw = divmod(t, 3)
                            lhsT = wb[:, 2 * t : 2 * t + 2, ocb * 128 : (ocb + 1) * 128]
                            rhs_slot = xp[:, 2 * n : 2 * n + 2, :]
                            for c in range(c0, c1):
                                off = c * ROWS * WPAD + kh * WPAD + kw
                                nc.tensor.matmul(
                                    psums[c - c0][:],
                                    lhsT,
                                    rhs_slot[:, :, off : off + NFREE],
                                    start=(t == 0),
                                    stop=(t == 8),
                                    perf_mode=DR,
                                )
                        for c in range(c0, c1):
                            out_c = out_pool.tile([128, ROWS, W], f32)
                            src = psums[c - c0].rearrange("p (h w) -> p h w", w=WPAD)[
                                :, :, 0:W
                            ]
                            # all drains on DVE (signs own ACT; Pool cannot
                            # read PSUM); the fp32 scale is applied here
                            nc.vector.tensor_scalar_mul(
                                out_c[:], src, sc[:, ocb : ocb + 1]
                            )
                            nc.sync.dma_start(
                                out=y[n, ocb * 128 : (ocb + 1) * 128, c * ROWS : (c + 1) * ROWS, :],
                                in_=out_c[:],
                            )

            def gate(next_tile, prev_tile):
                # The Tile scheduler hoists dependency-free dma_starts
                # ahead of emission order, so engine-queue position CANNOT
                # pace a transfer.  This tiny gpsimd copy READS the
                # previous piece's tile and WRITES into the next piece's
                # DMA destination: the next DMA then carries a real WAW
                # dependency and cannot start before the previous piece's
                # data has LANDED (gpsimd is otherwise idle, and the
                # garbage write is fully overwritten by the DMA).
                nc.gpsimd.tensor_copy(
                    next_tile[:, 0, 0, 0:2], prev_tile[:, 0, 0, 0:2]
                )

            # image 0: per-chunk pipeline.  Piece k+1's transfer is gated
            # on piece k's ARRIVAL (not its sign), so the ~2.6us piece
            # period beats the PE's 3.4us per chunk group with growing
            # slack.  Each single-chunk compute group is emitted right
            # after the one sign it needs.  Piece issues ride the sync
            # ring (out-DMAs only start later); signs own ACT.
            for k in range(NCHUNK):
                if k + 1 < NCHUNK:
                    r0, r1 = PR0[k + 1]
                    t = p_tiles[k + 1]
                    gate(t, p_tiles[k])
                    nc.sync.dma_start(
                        out=t[:, 0, 0 : r1 - r0, :], in_=x[0, 0:128, r0:r1, :]
                    )
                    nc.sync.dma_start(
                        out=t[:, 1, 0 : r1 - r0, :], in_=x[0, 128:256, r0:r1, :]
                    )
                sign_piece(0, k, p_tiles[k], PR0)
                compute_image(0, subs=((k, k + 1),))

            def load_image(n, gates):
                # images 1-3 ride the scalar ring; each piece's transfer
                # is gated on an earlier piece's arrival (2-piece
                # lookahead within the chain) so loads self-pace off the
                # startup window without stealing bandwidth.
                pad_ring(2 * n)
                pad_ring(2 * n + 1)
                tiles = []
                for p in range(len(PR)):
                    t = piece_tile(n, p)
                    tiles.append(t)
                    r0, r1 = PR[p]
                    gate(t, gates[p] if p < len(gates) else tiles[p - 2])
                    nc.scalar.dma_start(
                        out=t[:, 0, 0 : r1 - r0, :], in_=x[n, 0:128, r0:r1, :]
                    )
                    nc.scalar.dma_start(
                        out=t[:, 1, 0 : r1 - r0, :], in_=x[n, 128:256, r0:r1, :]
                    )
                    sign_piece(n, p, t, PR)
                return tiles

            i1 = load_image(1, gates=[p_tiles[3], p_tiles[5]])
            compute_image(1, subs=((0, 4),))
            i2 = load_image(2, gates=[i1[2], i1[3]])
            compute_image(1, subs=((4, NCHUNK),))
            compute_image(2, subs=((0, 4),))
            i3 = load_image(3, gates=[i2[2], i2[3]])
            compute_image(2, subs=((4, NCHUNK),))
            # final group is a single chunk so the drain+store tail after
            # the last matmul is as short as possible
            compute_image(3, subs=((0, 3), (3, 5), (5, 6), (6, NCHUNK)))

    _split_excess_waits(nc)
    return nc


def _get_nc():
    if "nc" not in _cache:
        _cache["nc"] = build_nc()
    return _cache["nc"]


def _prep_weights(weight, scale):
    # host-side: binarize weights, lay out [p, (kh kw icb), oc] fp8; the
    # per-channel scale is rearranged to [p, ocb].
    w = np.asarray(weight, dtype=np.float32)  # [oc, ic, kh, kw]
    wb = np.sign(w).transpose(2, 3, 1, 0)  # [kh, kw, ic, oc]
    wb = wb.reshape(3, 3, 2, 128, OC).transpose(3, 0, 1, 2, 4).reshape(128, 18, OC)
    wb8 = np.ascontiguousarray(wb).astype(ml_dtypes.float8_e4m3)
    sc2 = np.ascontiguousarray(np.asarray(scale, dtype=np.float32).reshape(2, 128).T)
    return wb8, sc2


def run(inputs, trace=False, trace_cores=None):
    from concourse.bass_utils import run_bass_kernel_spmd

    x = np.asarray(inputs["x"])
    wb8, sc2 = _prep_weights(inputs["weight"], inputs["scale"])

    in_maps = [
        {"x": x[i * IMGS : (i + 1) * IMGS], "wb8": wb8, "sc2": sc2}
        for i in range(N_CORES)
    ]
    res = run_bass_kernel_spmd(
        _get_nc(),
        in_maps,
        core_ids=list(range(N_CORES)),
        trace=trace,
        trace_cores=trace_cores,
    )
    out = np.concatenate([res.results[i]["y"] for i in range(N_CORES)], axis=0)
    return out, res


def kernel(**inputs):
    # One retry: a previously crashed process can leave a core wedged
    # (NRT_EXEC_UNIT_UNRECOVERABLE); the runtime recovers on the next
    # attempt.
    try:
        out, _ = run(inputs, trace=False)
    except Exception:
        out, _ = run(inputs, trace=False)
    return out
